# revision 29
# baseline (speedup 1.0000x reference)
"""AttentionBlock (GroupNorm + 1x1-conv QKV + full NxN attention + proj +
residual) on 8 Trainium2 NeuronCores, data-parallel over the batch dim.

Per core: 2 samples of x[16, 512, 32, 32]. Matmuls run in fp8e4m3 with
DoubleRow perf mode (128x256 virtual PE array, 0.5 cycles/row). PSUM
accumulation and the residual path stay fp32.

Key structural points vs a direct lowering:
  - GroupNorm affine folded into the QKV weights (host, exact).
  - K path eliminated: S^T = xn^T (M xn + bb) with M = Wk^T Wq and
    bb = Wk^T bq precomputed on host. The per-i and constant bias terms
    of S cancel in softmax; the per-j term is carried by bb folded into
    the Qm evacuation bias. Saves all K matmuls + K evacuations.
  - Weight-stationary reuse: matmuls are ordered so consecutive matmuls
    share one LDWEIGHTS; a post-legalization pass (_dedup_ldweights)
    removes the redundant loads tile_legalize inserts 1:1.
  - PSUM tiles are [128,1024] (2 banks) where possible so every
    evacuation instruction covers 1024 columns.
  - Softmax denominator: Z row-sums via ones-matmuls, 1/Z = exp(-ln Z)
    on ACT, broadcast across partitions with a K=1 bf16 matmul, folded
    into the O evacuation (DVE multiply).
  - proj bias + V bias + residual fused into one scalar_tensor_tensor
    per output tile: out = (proj_psum + pb') + x.
Engine budget per sample: PE ~37k cycles; ACT: exps/Qm-evac/V-evac/
lnz/rz/zb-copy; DVE: bn_stats/apply/ov/ob. GPSIMD only triggers DMAs
(Q7 tensor ops measured 14x slower than DVE - unusable).
"""

import math
import sys

import numpy as np

try:
    import concourse.bass as bass
except ImportError:  # pragma: no cover - grading container path setup
    sys.path.insert(0, "/opt/trn_rl_repo")
    import concourse.bass as bass

import bass_rust
import ml_dtypes
import concourse.tile as tile
from concourse import mybir
from concourse.bass_utils import run_bass_kernel_spmd

F32 = mybir.dt.float32
BF16 = mybir.dt.bfloat16
FP8 = mybir.dt.float8e4
DR = mybir.MatmulPerfMode.DoubleRow
AF = mybir.ActivationFunctionType
OP = mybir.AluOpType

NCORES = 8
B = 16
S = B // NCORES  # samples per core
C = 512
N = 1024  # H*W
G = 8  # groups
EPS = 1e-5
CT = C // 128  # channel p-tiles (4)
NT = N // 128  # spatial p-tiles (8)
IBS = 512  # i-block size
IB = N // IBS  # i blocks (2)
QS = 128.0  # host scale on M/bb so fp8 quantization has range
SEXP = 1.0 / (math.sqrt(C) * QS)  # exp() scale undoing QS

# Settable by test harness for profiling; not used by the grader.
TRACE = False
LAST_RESULT = None


MAX_WAITS = 1


def _split_excess_waits(nc, max_waits=MAX_WAITS):
    """Workaround for a walrus codegen limit: an instruction may carry at
    most `max_waits` semaphore waits ("Too many sync wait commands").
    Move the excess onto a chain of NOPs on the same engine right before
    the instruction — sequentially blocking waits on one engine queue are
    semantically identical to one multi-wait instruction."""
    counter = 0
    for f in nc.m.functions:
        for blk in f.blocks:
            il = blk.instructions
            if not any(
                i.sync_info is not None and len(i.sync_info.on_wait) > max_waits
                for i in il
            ):
                continue
            old = list(il)
            il.clear()
            for ins in old:
                si = ins.sync_info
                waits = list(si.on_wait) if si is not None else []
                if len(waits) > max_waits:
                    excess, keep = waits[:-max_waits], waits[-max_waits:]
                    for i0 in range(0, len(excess), max_waits):
                        counter += 1
                        nop = mybir.InstNoOp(
                            name=f"waitsplit-{counter}",
                            engine=ins.engine,
                            ins=[],
                            outs=[],
                            sync_info=bass_rust.SyncInfo(
                                on_wait=excess[i0 : i0 + max_waits], on_update=[]
                            ),
                        )
                        nc.register_instruction(nop, overwrite=True)
                        blk.add_instruction(nop)
                    ins.sync_info = bass_rust.SyncInfo(
                        on_wait=keep, on_update=list(si.on_update)
                    )
                blk.add_instruction(ins)
    return counter


def _dedup_ldweights(nc):
    """Remove InstLdweights that reload the PE-stationary already loaded.

    tile_legalize splits every matmul into LDWEIGHTS + MATMUL; in DR mode
    the load costs as much as the matmul itself, so matmuls are emitted
    grouped by stationary operand and the redundant loads removed here.
    The deleted load's waits move to the next PE instruction (same-queue
    blocking wait, identical semantics); its updates move to the previous
    PE instruction (the weights SBUF tile is not re-read once the array
    holds the data, so releasing it earlier is safe)."""
    removed = 0
    for f in nc.m.functions:
        for blk in f.blocks:
            il = list(blk.instructions)
            pe_idx = [
                i
                for i, ins in enumerate(il)
                if ins.engine == mybir.EngineType.PE
            ]
            cur_sig = None
            drop = set()
            for k, i in enumerate(pe_idx):
                ins = il[i]
                if isinstance(ins, mybir.InstLdweights):
                    sig = (
                        str(ins.ins[0]),
                        str(ins.perf_mode),
                        str(ins.is_transpose),
                        str(ins.tile_position),
                        str(ins.tile_size),
                    )
                    if sig == cur_sig:
                        # move waits -> next PE inst, updates -> prev PE inst
                        si = ins.sync_info
                        if si is not None and (si.on_wait or si.on_update):
                            if si.on_wait:
                                nxt = il[pe_idx[k + 1]]
                                nsi = nxt.sync_info
                                nw = list(nsi.on_wait) if nsi else []
                                nu = list(nsi.on_update) if nsi else []
                                nxt.sync_info = bass_rust.SyncInfo(
                                    on_wait=list(si.on_wait) + nw, on_update=nu
                                )
                            if si.on_update:
                                prv = il[pe_idx[k - 1]]
                                psi = prv.sync_info
                                pw = list(psi.on_wait) if psi else []
                                pu = list(psi.on_update) if psi else []
                                prv.sync_info = bass_rust.SyncInfo(
                                    on_wait=pw, on_update=pu + list(si.on_update)
                                )
                        drop.add(i)
                        removed += 1
                    else:
                        cur_sig = sig
                elif isinstance(ins, mybir.InstMatmult):
                    if ins.is_transpose:
                        cur_sig = None
                else:
                    # semaphore/nop/branch etc don't touch the PE array
                    pass
            if drop:
                blk.instructions.clear()
                for i, ins in enumerate(il):
                    if i not in drop:
                        blk.add_instruction(ins)
    return removed


def _build():
    from contextlib import ExitStack

    nc = bass.Bass()
    xd = nc.declare_dram_parameter("x", [S, C, N], F32, isOutput=False)
    wald = nc.declare_dram_parameter("wall", [128, 3 * CT, C], FP8, isOutput=False)
    bald = nc.declare_dram_parameter("ball", [128, 2 * CT], F32, isOutput=False)
    cad = nc.declare_dram_parameter("ca", [128, 2], BF16, isOutput=False)
    cbd = nc.declare_dram_parameter("cb", [2, 128], BF16, isOutput=False)
    outd = nc.declare_dram_parameter("out", [S, C, N], BF16, isOutput=True)

    x_ap = xd[:].rearrange("s (t p) n -> s t p n", p=128)
    out_ap = outd[:].rearrange("s (t p) n -> s p t n", p=128)

    with tile.TileContext(nc) as tc, ExitStack() as ctx:
        singles = ctx.enter_context(tc.tile_pool(name="singles", bufs=1))
        xp = ctx.enter_context(tc.tile_pool(name="xp", bufs=S))
        xnp = ctx.enter_context(tc.tile_pool(name="xnp", bufs=2 * S))
        qp = ctx.enter_context(tc.tile_pool(name="qp", bufs=2))
        vp = ctx.enter_context(tc.tile_pool(name="vp", bufs=2))
        ep = ctx.enter_context(tc.tile_pool(name="ep", bufs=12))
        op_ = ctx.enter_context(tc.tile_pool(name="op", bufs=2))
        rzp = ctx.enter_context(tc.tile_pool(name="rzp", bufs=2))
        zbp = ctx.enter_context(tc.tile_pool(name="zbp", bufs=2))
        obp = ctx.enter_context(tc.tile_pool(name="obp", bufs=3))
        smp = ctx.enter_context(tc.tile_pool(name="smp", bufs=4))
        ps = ctx.enter_context(tc.tile_pool(name="ps", bufs=1, space="PSUM"))

        # ---- input DMAs: constants first (tiny), then x tiles ----
        ball = singles.tile([128, 2 * CT], F32)
        nc.gpsimd.dma_start(out=ball[:], in_=bald[:])
        ca = singles.tile([128, 2], BF16)
        nc.gpsimd.dma_start(out=ca[:], in_=cad[:])
        cb = singles.tile([2, 128], BF16)
        nc.gpsimd.dma_start(out=cb[:], in_=cbd[:])
        eps_sb = singles.tile([2, 1], F32)
        nc.vector.memset(eps_sb[:], EPS)
        ones8 = singles.tile([128, 2, 16], FP8)
        nc.vector.memset(ones8[:], 1.0)
        ones_row = singles.tile([1, 128], BF16)
        nc.vector.memset(ones_row[:], 1.0)
        xs = []
        wall = singles.tile([128, 3 * CT, C], FP8)
        wall_v = wall.rearrange("p (w r) f -> p w r f", w=3)
        wald_v = wald[:].rearrange("p (w r) f -> p w r f", w=3)
        nc.sync.dma_start(out=wall_v[:, 0], in_=wald_v[:, 0])
        for s in range(S):
            x_sb = xp.tile([128, CT, N], F32, tag="x")
            xs.append(x_sb)
            for t in range(CT):
                eng = nc.sync if t % 2 == 0 else nc.scalar
                eng.dma_start(out=x_sb[:, t, :], in_=x_ap[s, t])
            if s == S - 1:
                nc.sync.dma_start(out=wall_v[:, 1], in_=wald_v[:, 1])
                nc.scalar.dma_start(out=wall_v[:, 2], in_=wald_v[:, 2])
        # weight planes for DoubleRow: [p, wi, g, q, o]; channel = 256g+128q+p
        w8 = wall.rearrange("p (w g q) f -> p w g q f", g=2, q=2)
        bb_sb, pb = ball[:, 0:CT], ball[:, CT : 2 * CT]
        ghot, hhot = ca[:, 0:2], cb[:, :]

        xns = [None] * S
        sas = [None] * S

        def emit_gn_stats(s, h0):
            """Group stats for tile pair h0. s0 h1 goes on ACT (DVE does
            h0 + chain work); sample 1 is throughput-bound -> all DVE."""
            x_sb = xs[s]
            if h0 == 0:
                xn_g = [
                    xnp.tile([128, 2, N], FP8, tag="xn", name=f"xn{s}{g}")
                    for g in range(2)
                ]
                xns[s] = xn_g
                sa_h = [
                    smp.tile([128, 4], BF16, tag="sa", name=f"sa{s}{h}", bufs=2 * S)
                    for h in range(2)
                ]
                sas[s] = sa_h
            sa_h = sas[s]

            def act_stats(t):
                sa = sa_h[t // 2]
                scr = smp.tile([128, N], BF16, tag="scr", bufs=2)
                sum3 = smp.tile([128, 1], F32, tag="sum3", bufs=2)
                nc.scalar.activation(
                    out=scr[:], in_=x_sb[:, t, :], func=AF.Identity,
                    accum_out=sum3[:],
                )
                scr2 = smp.tile([128, N], BF16, tag="scr", bufs=2)
                sq3 = smp.tile([128, 1], F32, tag="sq3", bufs=2)
                nc.scalar.activation(
                    out=scr2[:], in_=x_sb[:, t, :], func=AF.Square,
                    accum_out=sq3[:],
                )
                c0 = 2 * (t % 2)
                nc.scalar.activation(
                    out=sa[:, c0 : c0 + 1], in_=sum3[:], func=AF.Identity,
                    scale=1.0 / N,
                )
                nc.scalar.activation(
                    out=sa[:, c0 + 1 : c0 + 2], in_=sq3[:], func=AF.Identity,
                    scale=1.0 / N,
                )

            def dve_stats(t):
                sa = sa_h[t // 2]
                st6 = smp.tile([128, 2, 6], F32, tag="st6", bufs=2)
                nc.vector.bn_stats(out=st6[:, 0, :], in_=x_sb[:, t, 0:512])
                nc.vector.bn_stats(out=st6[:, 1, :], in_=x_sb[:, t, 512:1024])
                mv = smp.tile([128, 2], F32, tag="mv", bufs=2)
                nc.vector.bn_aggr(out=mv[:], in_=st6[:])
                c0 = 2 * (t % 2)
                nc.vector.tensor_copy(out=sa[:, c0 : c0 + 1], in_=mv[:, 0:1])
                msq = smp.tile([128, 1], F32, tag="msq", bufs=2)
                nc.vector.tensor_mul(msq[:], mv[:, 0:1], mv[:, 0:1])
                nc.vector.tensor_tensor(
                    out=sa[:, c0 + 1 : c0 + 2], in0=mv[:, 1:2], in1=msq[:],
                    op=OP.add,
                )

            if h0 == 0:
                dve_stats(0)
                dve_stats(1)
            else:
                if s == 0:
                    act_stats(2)
                    act_stats(3)
                else:
                    dve_stats(2)
                    dve_stats(3)

        def emit_gn_chain(s, h0):
            """Group reduce -> rstd -> broadcast -> apply for tile pair h0."""
            x_sb = xs[s]
            xn_g = xns[s]
            gs_ps = ps.tile([2, 4], F32, tag="sm", bufs=2)
            nc.tensor.matmul(
                gs_ps[:], lhsT=ghot, rhs=sas[s][h0][:], start=True, stop=True
            )
            gs3 = gs_ps.rearrange("h (t s) -> h t s", s=2)
            sq = smp.tile([2, 2], F32, tag="sq")
            nc.scalar.activation(out=sq[:], in_=gs3[:, :, 0], func=AF.Square)
            var = smp.tile([2, 2], F32, tag="var")
            nc.vector.tensor_tensor(
                out=var[:], in0=gs3[:, :, 1], in1=sq[:], op=OP.subtract
            )
            lnv = smp.tile([2, 2], F32, tag="lnv")
            nc.scalar.activation(
                out=lnv[:], in_=var[:], func=AF.Ln, bias=eps_sb[:], scale=1.0
            )
            # vals: (rstd, mean*rstd); apply is x*rstd - mean*rstd
            vals = smp.tile([2, 4], BF16, tag="vals")
            vals3 = vals.rearrange("h (t s) -> h t s", s=2)
            nc.scalar.activation(
                out=vals3[:, :, 0], in_=lnv[:], func=AF.Exp, scale=-0.5
            )
            nc.vector.scalar_tensor_tensor(
                out=vals3[:, :, 1], in0=gs3[:, :, 0], scalar=-1.0,
                in1=vals3[:, :, 0], op0=OP.mult, op1=OP.mult,
            )
            bc = ps.tile([128, 4], F32, tag="sm", bufs=2)
            nc.tensor.matmul(bc[:], lhsT=hhot, rhs=vals[:], start=True, stop=True)
            bcs = smp.tile([128, 4], F32, tag="bcs", bufs=2)
            nc.vector.tensor_copy(out=bcs[:], in_=bc[:])
            for tt in range(2):
                t = 2 * h0 + tt
                if tt == 0:
                    nc.scalar.activation(
                        out=xn_g[t // 2][:, t % 2, :],
                        in_=x_sb[:, t, :],
                        func=AF.Identity,
                        scale=bcs[:, 2 * tt : 2 * tt + 1],
                        bias=bcs[:, 2 * tt + 1 : 2 * tt + 2],
                    )
                else:
                    nc.vector.tensor_scalar(
                        out=xn_g[t // 2][:, t % 2, :],
                        in0=x_sb[:, t, :],
                        scalar1=bcs[:, 2 * tt : 2 * tt + 1],
                        scalar2=bcs[:, 2 * tt + 1 : 2 * tt + 2],
                        op0=OP.mult,
                        op1=OP.add,
                    )

        qs = [None] * S

        def emit_qm(s):
            """Qm = QS*(M xn) per output tile; evac on ACT adds QS*bb."""
            xn_g = xns[s]
            q_sb = qp.tile([128, 2, 2, N], FP8, tag="q")
            qs[s] = q_sb
            for ot in range(CT):
                psm = ps.tile([128, N], F32, tag="mm2", bufs=3)
                for g in range(2):
                    for ib in range(IB):
                        nc.tensor.matmul(
                            psm[:, ib * IBS : (ib + 1) * IBS],
                            lhsT=w8[:, 0, g, :, ot * 128 : (ot + 1) * 128],
                            rhs=xn_g[g][:, :, ib * IBS : (ib + 1) * IBS],
                            start=(g == 0),
                            stop=(g == 1),
                            perf_mode=DR,
                        )
                if ot % 2 == 0:
                    nc.scalar.activation(
                        out=q_sb[:, ot // 2, ot % 2, :],
                        in_=psm[:],
                        func=AF.Identity,
                        bias=bb_sb[:, ot : ot + 1],
                        scale=1.0,
                    )
                else:
                    nc.vector.tensor_scalar(
                        out=q_sb[:, ot // 2, ot % 2, :],
                        in0=psm[:],
                        scalar1=bb_sb[:, ot : ot + 1],
                        scalar2=None,
                        op0=OP.add,
                    )

        vs = [None] * S
        ess = [None] * S

        def emit_sv(s):
            """Fused S^T/V^T: per (nt, g) one xn stationary serves both."""
            xn_g = xns[s]
            q_sb = qs[s]
            v_sb = vp.tile([128, NT // 2, 2, C], FP8, tag="v")
            vs[s] = v_sb
            es = [
                ep.tile([128, 2, 2, IBS], FP8, tag="e", name=f"e{s}{jg}")
                for jg in range(NT // 2)
            ]
            ess[s] = es
            psm_v = None
            for nt in range(NT):
                psm_s = ps.tile([128, N], F32, tag="mm2", bufs=3)
                if nt % 2 == 0:
                    psm_v = ps.tile(
                        [128, 2, C], F32, tag="mm2", bufs=3, name=f"psmv{s}{nt}"
                    )
                for g in range(2):
                    lw = xn_g[g][:, :, nt * 128 : (nt + 1) * 128]
                    s_mm = None
                    for ib in range(IB):
                        s_mm = nc.tensor.matmul(
                            psm_s[:, ib * IBS : (ib + 1) * IBS],
                            lhsT=lw,
                            rhs=q_sb[:, g, :, ib * IBS : (ib + 1) * IBS],
                            start=(g == 0),
                            stop=(g == 1),
                            perf_mode=DR,
                        )
                    nc.tensor.matmul(
                        psm_v[:, nt % 2, :],
                        lhsT=lw,
                        rhs=w8[:, 1, g, :, :],
                        start=(g == 0),
                        stop=(g == 1),
                        perf_mode=DR,
                    )
                nc.scalar.activation(
                    out=es[nt // 2][:, nt % 2, :, :],
                    in_=psm_s[:],
                    func=AF.Exp,
                    scale=SEXP,
                )
                # V evacuation on DVE, one [128,1024] op per nt pair
                if nt % 2 == 1:
                    nc.vector.tensor_copy(
                        out=v_sb[:, nt // 2, :, :], in_=psm_v[:]
                    )

        def emit_zop(s):
            """Z row-sums, 1/Z, broadcast, O = V@E^T, proj + residual."""
            x_sb = xs[s]
            v_sb = vs[s]
            es = ess[s]
            # Z
            rz = rzp.tile([1, N], BF16, tag="rz")
            for ib in range(IB):
                zps = ps.tile([1, IBS], F32, tag="sm", bufs=2)
                for jg in range(NT // 2):
                    nc.tensor.matmul(
                        zps[:],
                        lhsT=ones8[:, :, 0:1],
                        rhs=es[jg][:, :, ib, :],
                        start=(jg == 0),
                        stop=(jg == NT // 2 - 1),
                        perf_mode=DR,
                    )
                lnz = rzp.tile([1, IBS], F32, tag="lnz", bufs=2)
                nc.scalar.activation(out=lnz[:], in_=zps[:], func=AF.Ln)
                nc.scalar.activation(
                    out=rz[:, ib * IBS : (ib + 1) * IBS], in_=lnz[:],
                    func=AF.Exp, scale=-1.0,
                )
            # broadcast 1/Z across partitions (bf16 K=1 matmul) -> zb
            zb_ps = ps.tile([128, N], F32, tag="mm2", bufs=3)
            for ib in range(IB):
                nc.tensor.matmul(
                    zb_ps[:, ib * IBS : (ib + 1) * IBS],
                    lhsT=ones_row,
                    rhs=rz[:, ib * IBS : (ib + 1) * IBS],
                    start=True,
                    stop=True,
                )
            zb = zbp.tile([128, N], BF16, tag="zb")
            nc.scalar.activation(out=zb[:], in_=zb_ps[:], func=AF.Identity)
            # O = V @ E^T, normalized during evacuation
            o_sb = op_.tile([128, 2, 2, N], FP8, tag="o")
            for ct in range(CT):
                psm_o = ps.tile([128, N], F32, tag="mm2", bufs=3)
                for jg in range(NT // 2):
                    for ib in range(IB):
                        nc.tensor.matmul(
                            psm_o[:, ib * IBS : (ib + 1) * IBS],
                            lhsT=v_sb[:, jg, :, ct * 128 : (ct + 1) * 128],
                            rhs=es[jg][:, :, ib, :],
                            start=(jg == 0),
                            stop=(jg == NT // 2 - 1),
                            perf_mode=DR,
                        )
                nc.vector.tensor_tensor(
                    out=o_sb[:, ct // 2, ct % 2, :], in0=psm_o[:], in1=zb[:],
                    op=OP.mult,
                )
            # proj + pb + residual, one op per output tile, then DMA out
            for ot in range(CT):
                psm_p = ps.tile([128, N], F32, tag="mm2", bufs=3)
                for g in range(2):
                    for ib in range(IB):
                        nc.tensor.matmul(
                            psm_p[:, ib * IBS : (ib + 1) * IBS],
                            lhsT=w8[:, 2, g, :, ot * 128 : (ot + 1) * 128],
                            rhs=o_sb[:, g, :, ib * IBS : (ib + 1) * IBS],
                            start=(g == 0),
                            stop=(g == 1),
                            perf_mode=DR,
                        )
                ob = obp.tile([128, N], BF16, tag="ob")
                nc.vector.scalar_tensor_tensor(
                    out=ob[:],
                    in0=psm_p[:],
                    scalar=pb[:, ot : ot + 1],
                    in1=x_sb[:, ot, :],
                    op0=OP.add,
                    op1=OP.add,
                )
                for hh in range(2):
                    eng = nc.sync if (2 * ot + hh) % 2 == 0 else nc.scalar
                    eng.dma_start(
                        out=out_ap[s][:, ot, hh * IBS : (hh + 1) * IBS],
                        in_=ob[:, hh * IBS : (hh + 1) * IBS],
                    )

        emit_gn_stats(0, 0)
        emit_gn_chain(0, 0)
        emit_gn_stats(0, 1)
        emit_gn_chain(0, 1)
        emit_qm(0)
        emit_gn_stats(1, 0)
        emit_gn_stats(1, 1)
        emit_sv(0)
        emit_gn_chain(1, 0)
        emit_gn_chain(1, 1)
        emit_qm(1)
        emit_sv(1)
        emit_zop(0)
        emit_zop(1)

    _dedup_ldweights(nc)
    _split_excess_waits(nc)
    return nc


_NC = None


def kernel(x, norm_w, norm_b, qkv_w, qkv_b, proj_w, proj_b):
    global _NC, LAST_RESULT
    x = np.ascontiguousarray(np.asarray(x, dtype=np.float32))
    norm_w = np.asarray(norm_w, dtype=np.float32)
    norm_b = np.asarray(norm_b, dtype=np.float32)
    qkv_w = np.asarray(qkv_w, dtype=np.float32)
    qkv_b = np.asarray(qkv_b, dtype=np.float32)
    proj_w = np.asarray(proj_w, dtype=np.float32)
    proj_b = np.asarray(proj_b, dtype=np.float32)

    # fold GroupNorm affine into qkv
    wq_full = qkv_w * norm_w[None, :]
    bq_full = qkv_b + qkv_w @ norm_b
    wq_, wk_, wv_ = wq_full[0:C], wq_full[C : 2 * C], wq_full[2 * C : 3 * C]
    bq_, bk_, bv_ = bq_full[0:C], bq_full[C : 2 * C], bq_full[2 * C : 3 * C]
    del bk_  # cancels in softmax (per-i and constant terms)
    pb_ = proj_w @ bv_ + proj_b
    m_ = wk_.T @ wq_  # S^T = xn^T (M xn + bb); QS scales into fp8 range
    bb_ = wk_.T @ bq_

    def wtile(w):  # [o, c] -> DoubleRow lhsT planes [128, 2(g), 2(q), o]
        return w.T.reshape(2, 2, 128, C).transpose(2, 0, 1, 3)

    def btile(b):  # [C] -> [128, ct]
        return b.reshape(CT, 128).T

    wall = np.ascontiguousarray(
        np.stack(
            [wtile(m_ * QS), wtile(wv_), wtile(proj_w)], axis=1
        ).reshape(128, 12, C).astype(ml_dtypes.float8_e4m3)
    )
    ball = np.ascontiguousarray(
        np.concatenate([btile(bb_ * QS), btile(pb_)], axis=1).astype(np.float32)
    )
    cl = np.arange(128)
    ghot = np.zeros((128, 2), np.float32)
    ghot[cl, cl // 64] = 1.0 / 64.0
    hhot = np.zeros((2, 128), np.float32)
    hhot[cl // 64, cl] = 1.0

    common = {
        "wall": wall,
        "ball": ball,
        "ca": ghot.astype(ml_dtypes.bfloat16),
        "cb": hhot.astype(ml_dtypes.bfloat16),
    }
    xr = x.reshape(NCORES, S, C, N)
    in_maps = [dict(common, x=np.ascontiguousarray(xr[i])) for i in range(NCORES)]

    if _NC is None:
        _NC = _build()
    res = run_bass_kernel_spmd(
        _NC, in_maps, core_ids=list(range(NCORES)), trace=TRACE
    )
    LAST_RESULT = res
    out = np.stack([res.results[i]["out"] for i in range(NCORES)])
    return np.ascontiguousarray(out.reshape(B, C, 32, 32).astype(np.float32))


# revision 30
# speedup vs baseline: 1.0004x; 1.0004x over previous
"""AttentionBlock (GroupNorm + 1x1-conv QKV + full NxN attention + proj +
residual) on 8 Trainium2 NeuronCores, data-parallel over the batch dim.

Per core: 2 samples of x[16, 512, 32, 32]. Matmuls run in fp8e4m3 with
DoubleRow perf mode (128x256 virtual PE array, 0.5 cycles/row). PSUM
accumulation and the residual path stay fp32.

Key structural points vs a direct lowering:
  - GroupNorm affine folded into the QKV weights (host, exact).
  - K path eliminated: S^T = xn^T (M xn + bb) with M = Wk^T Wq and
    bb = Wk^T bq precomputed on host. The per-i and constant bias terms
    of S cancel in softmax; the per-j term is carried by bb folded into
    the Qm evacuation bias. Saves all K matmuls + K evacuations.
  - Weight-stationary reuse: matmuls are ordered so consecutive matmuls
    share one LDWEIGHTS; a post-legalization pass (_dedup_ldweights)
    removes the redundant loads tile_legalize inserts 1:1.
  - PSUM tiles are [128,1024] (2 banks) where possible so every
    evacuation instruction covers 1024 columns.
  - Softmax denominator: Z row-sums via ones-matmuls, 1/Z = exp(-ln Z)
    on ACT, broadcast across partitions with a K=1 bf16 matmul, folded
    into the O evacuation (DVE multiply).
  - proj bias + V bias + residual fused into one scalar_tensor_tensor
    per output tile: out = (proj_psum + pb') + x.
Engine budget per sample: PE ~37k cycles; ACT: exps/Qm-evac/V-evac/
lnz/rz/zb-copy; DVE: bn_stats/apply/ov/ob. GPSIMD only triggers DMAs
(Q7 tensor ops measured 14x slower than DVE - unusable).
"""

import math
import sys

import numpy as np

try:
    import concourse.bass as bass
except ImportError:  # pragma: no cover - grading container path setup
    sys.path.insert(0, "/opt/trn_rl_repo")
    import concourse.bass as bass

import bass_rust
import ml_dtypes
import concourse.tile as tile
from concourse import mybir
from concourse.bass_utils import run_bass_kernel_spmd

F32 = mybir.dt.float32
BF16 = mybir.dt.bfloat16
FP8 = mybir.dt.float8e4
DR = mybir.MatmulPerfMode.DoubleRow
AF = mybir.ActivationFunctionType
OP = mybir.AluOpType

NCORES = 8
B = 16
S = B // NCORES  # samples per core
C = 512
N = 1024  # H*W
G = 8  # groups
EPS = 1e-5
CT = C // 128  # channel p-tiles (4)
NT = N // 128  # spatial p-tiles (8)
IBS = 512  # i-block size
IB = N // IBS  # i blocks (2)
QS = 128.0  # host scale on M/bb so fp8 quantization has range
SEXP = 1.0 / (math.sqrt(C) * QS)  # exp() scale undoing QS

# Settable by test harness for profiling; not used by the grader.
TRACE = False
LAST_RESULT = None


MAX_WAITS = 1


def _split_excess_waits(nc, max_waits=MAX_WAITS):
    """Workaround for a walrus codegen limit: an instruction may carry at
    most `max_waits` semaphore waits ("Too many sync wait commands").
    Move the excess onto a chain of NOPs on the same engine right before
    the instruction — sequentially blocking waits on one engine queue are
    semantically identical to one multi-wait instruction."""
    counter = 0
    for f in nc.m.functions:
        for blk in f.blocks:
            il = blk.instructions
            if not any(
                i.sync_info is not None and len(i.sync_info.on_wait) > max_waits
                for i in il
            ):
                continue
            old = list(il)
            il.clear()
            for ins in old:
                si = ins.sync_info
                waits = list(si.on_wait) if si is not None else []
                if len(waits) > max_waits:
                    excess, keep = waits[:-max_waits], waits[-max_waits:]
                    for i0 in range(0, len(excess), max_waits):
                        counter += 1
                        nop = mybir.InstNoOp(
                            name=f"waitsplit-{counter}",
                            engine=ins.engine,
                            ins=[],
                            outs=[],
                            sync_info=bass_rust.SyncInfo(
                                on_wait=excess[i0 : i0 + max_waits], on_update=[]
                            ),
                        )
                        nc.register_instruction(nop, overwrite=True)
                        blk.add_instruction(nop)
                    ins.sync_info = bass_rust.SyncInfo(
                        on_wait=keep, on_update=list(si.on_update)
                    )
                blk.add_instruction(ins)
    return counter


def _dedup_ldweights(nc):
    """Remove InstLdweights that reload the PE-stationary already loaded.

    tile_legalize splits every matmul into LDWEIGHTS + MATMUL; in DR mode
    the load costs as much as the matmul itself, so matmuls are emitted
    grouped by stationary operand and the redundant loads removed here.
    The deleted load's waits move to the next PE instruction (same-queue
    blocking wait, identical semantics); its updates move to the previous
    PE instruction (the weights SBUF tile is not re-read once the array
    holds the data, so releasing it earlier is safe)."""
    removed = 0
    for f in nc.m.functions:
        for blk in f.blocks:
            il = list(blk.instructions)
            pe_idx = [
                i
                for i, ins in enumerate(il)
                if ins.engine == mybir.EngineType.PE
            ]
            cur_sig = None
            drop = set()
            for k, i in enumerate(pe_idx):
                ins = il[i]
                if isinstance(ins, mybir.InstLdweights):
                    sig = (
                        str(ins.ins[0]),
                        str(ins.perf_mode),
                        str(ins.is_transpose),
                        str(ins.tile_position),
                        str(ins.tile_size),
                    )
                    if sig == cur_sig:
                        # move waits -> next PE inst, updates -> prev PE inst
                        si = ins.sync_info
                        if si is not None and (si.on_wait or si.on_update):
                            if si.on_wait:
                                nxt = il[pe_idx[k + 1]]
                                nsi = nxt.sync_info
                                nw = list(nsi.on_wait) if nsi else []
                                nu = list(nsi.on_update) if nsi else []
                                nxt.sync_info = bass_rust.SyncInfo(
                                    on_wait=list(si.on_wait) + nw, on_update=nu
                                )
                            if si.on_update:
                                prv = il[pe_idx[k - 1]]
                                psi = prv.sync_info
                                pw = list(psi.on_wait) if psi else []
                                pu = list(psi.on_update) if psi else []
                                prv.sync_info = bass_rust.SyncInfo(
                                    on_wait=pw, on_update=pu + list(si.on_update)
                                )
                        drop.add(i)
                        removed += 1
                    else:
                        cur_sig = sig
                elif isinstance(ins, mybir.InstMatmult):
                    if ins.is_transpose:
                        cur_sig = None
                else:
                    # semaphore/nop/branch etc don't touch the PE array
                    pass
            if drop:
                blk.instructions.clear()
                for i, ins in enumerate(il):
                    if i not in drop:
                        blk.add_instruction(ins)
    return removed


def _build():
    from contextlib import ExitStack

    nc = bass.Bass()
    xd = nc.declare_dram_parameter("x", [S, C, N], F32, isOutput=False)
    wald = nc.declare_dram_parameter("wall", [128, 3 * CT, C], FP8, isOutput=False)
    bald = nc.declare_dram_parameter("ball", [128, 2 * CT], F32, isOutput=False)
    cad = nc.declare_dram_parameter("ca", [128, 2], BF16, isOutput=False)
    cbd = nc.declare_dram_parameter("cb", [2, 128], BF16, isOutput=False)
    outd = nc.declare_dram_parameter("out", [S, C, N], BF16, isOutput=True)

    x_ap = xd[:].rearrange("s (t p) n -> s t p n", p=128)
    out_ap = outd[:].rearrange("s (t p) n -> s p t n", p=128)

    with tile.TileContext(nc) as tc, ExitStack() as ctx:
        singles = ctx.enter_context(tc.tile_pool(name="singles", bufs=1))
        xp = ctx.enter_context(tc.tile_pool(name="xp", bufs=S))
        xnp = ctx.enter_context(tc.tile_pool(name="xnp", bufs=2 * S))
        qp = ctx.enter_context(tc.tile_pool(name="qp", bufs=2))
        vp = ctx.enter_context(tc.tile_pool(name="vp", bufs=2))
        ep = ctx.enter_context(tc.tile_pool(name="ep", bufs=12))
        op_ = ctx.enter_context(tc.tile_pool(name="op", bufs=2))
        rzp = ctx.enter_context(tc.tile_pool(name="rzp", bufs=2))
        zbp = ctx.enter_context(tc.tile_pool(name="zbp", bufs=2))
        obp = ctx.enter_context(tc.tile_pool(name="obp", bufs=3))
        smp = ctx.enter_context(tc.tile_pool(name="smp", bufs=4))
        ps = ctx.enter_context(tc.tile_pool(name="ps", bufs=1, space="PSUM"))

        # ---- input DMAs: constants first (tiny), then x tiles ----
        ball = singles.tile([128, 2 * CT], F32)
        nc.gpsimd.dma_start(out=ball[:], in_=bald[:])
        ca = singles.tile([128, 2], BF16)
        nc.gpsimd.dma_start(out=ca[:], in_=cad[:])
        cb = singles.tile([2, 128], BF16)
        nc.gpsimd.dma_start(out=cb[:], in_=cbd[:])
        eps_sb = singles.tile([2, 1], F32)
        nc.vector.memset(eps_sb[:], EPS)
        ones8 = singles.tile([128, 2, 16], FP8)
        nc.vector.memset(ones8[:], 1.0)
        ones_row = singles.tile([1, 128], BF16)
        nc.vector.memset(ones_row[:], 1.0)
        xs = []
        wall = singles.tile([128, 3 * CT, C], FP8)
        wall_v = wall.rearrange("p (w r) f -> p w r f", w=3)
        wald_v = wald[:].rearrange("p (w r) f -> p w r f", w=3)
        nc.sync.dma_start(out=wall_v[:, 0], in_=wald_v[:, 0])
        for s in range(S):
            x_sb = xp.tile([128, CT, N], F32, tag="x")
            xs.append(x_sb)
            for t in range(CT):
                eng = nc.sync if t % 2 == 0 else nc.scalar
                eng.dma_start(out=x_sb[:, t, :], in_=x_ap[s, t])
            if s == 0:
                nc.sync.dma_start(out=wall_v[:, 1], in_=wald_v[:, 1])
                nc.scalar.dma_start(out=wall_v[:, 2], in_=wald_v[:, 2])
        # weight planes for DoubleRow: [p, wi, g, q, o]; channel = 256g+128q+p
        w8 = wall.rearrange("p (w g q) f -> p w g q f", g=2, q=2)
        bb_sb, pb = ball[:, 0:CT], ball[:, CT : 2 * CT]
        ghot, hhot = ca[:, 0:2], cb[:, :]

        xns = [None] * S
        sas = [None] * S

        def emit_gn_stats(s, h0):
            """Group stats for tile pair h0. s0 h1 goes on ACT (DVE does
            h0 + chain work); sample 1 is throughput-bound -> all DVE."""
            x_sb = xs[s]
            if h0 == 0:
                xn_g = [
                    xnp.tile([128, 2, N], FP8, tag="xn", name=f"xn{s}{g}")
                    for g in range(2)
                ]
                xns[s] = xn_g
                sa_h = [
                    smp.tile([128, 4], BF16, tag="sa", name=f"sa{s}{h}", bufs=2 * S)
                    for h in range(2)
                ]
                sas[s] = sa_h
            sa_h = sas[s]

            def act_stats(t):
                sa = sa_h[t // 2]
                scr = smp.tile([128, N], BF16, tag="scr", bufs=2)
                sum3 = smp.tile([128, 1], F32, tag="sum3", bufs=2)
                nc.scalar.activation(
                    out=scr[:], in_=x_sb[:, t, :], func=AF.Identity,
                    accum_out=sum3[:],
                )
                scr2 = smp.tile([128, N], BF16, tag="scr", bufs=2)
                sq3 = smp.tile([128, 1], F32, tag="sq3", bufs=2)
                nc.scalar.activation(
                    out=scr2[:], in_=x_sb[:, t, :], func=AF.Square,
                    accum_out=sq3[:],
                )
                c0 = 2 * (t % 2)
                nc.scalar.activation(
                    out=sa[:, c0 : c0 + 1], in_=sum3[:], func=AF.Identity,
                    scale=1.0 / N,
                )
                nc.scalar.activation(
                    out=sa[:, c0 + 1 : c0 + 2], in_=sq3[:], func=AF.Identity,
                    scale=1.0 / N,
                )

            def dve_stats(t):
                sa = sa_h[t // 2]
                st6 = smp.tile([128, 2, 6], F32, tag="st6", bufs=2)
                nc.vector.bn_stats(out=st6[:, 0, :], in_=x_sb[:, t, 0:512])
                nc.vector.bn_stats(out=st6[:, 1, :], in_=x_sb[:, t, 512:1024])
                mv = smp.tile([128, 2], F32, tag="mv", bufs=2)
                nc.vector.bn_aggr(out=mv[:], in_=st6[:])
                c0 = 2 * (t % 2)
                nc.vector.tensor_copy(out=sa[:, c0 : c0 + 1], in_=mv[:, 0:1])
                msq = smp.tile([128, 1], F32, tag="msq", bufs=2)
                nc.vector.tensor_mul(msq[:], mv[:, 0:1], mv[:, 0:1])
                nc.vector.tensor_tensor(
                    out=sa[:, c0 + 1 : c0 + 2], in0=mv[:, 1:2], in1=msq[:],
                    op=OP.add,
                )

            if h0 == 0:
                dve_stats(0)
                dve_stats(1)
            else:
                if s == 0:
                    act_stats(2)
                    act_stats(3)
                else:
                    dve_stats(2)
                    dve_stats(3)

        def emit_gn_chain(s, h0):
            """Group reduce -> rstd -> broadcast -> apply for tile pair h0."""
            x_sb = xs[s]
            xn_g = xns[s]
            gs_ps = ps.tile([2, 4], F32, tag="sm", bufs=2)
            nc.tensor.matmul(
                gs_ps[:], lhsT=ghot, rhs=sas[s][h0][:], start=True, stop=True
            )
            gs3 = gs_ps.rearrange("h (t s) -> h t s", s=2)
            sq = smp.tile([2, 2], F32, tag="sq")
            nc.scalar.activation(out=sq[:], in_=gs3[:, :, 0], func=AF.Square)
            var = smp.tile([2, 2], F32, tag="var")
            nc.vector.tensor_tensor(
                out=var[:], in0=gs3[:, :, 1], in1=sq[:], op=OP.subtract
            )
            lnv = smp.tile([2, 2], F32, tag="lnv")
            nc.scalar.activation(
                out=lnv[:], in_=var[:], func=AF.Ln, bias=eps_sb[:], scale=1.0
            )
            # vals: (rstd, mean*rstd); apply is x*rstd - mean*rstd
            vals = smp.tile([2, 4], BF16, tag="vals")
            vals3 = vals.rearrange("h (t s) -> h t s", s=2)
            nc.scalar.activation(
                out=vals3[:, :, 0], in_=lnv[:], func=AF.Exp, scale=-0.5
            )
            nc.vector.scalar_tensor_tensor(
                out=vals3[:, :, 1], in0=gs3[:, :, 0], scalar=-1.0,
                in1=vals3[:, :, 0], op0=OP.mult, op1=OP.mult,
            )
            bc = ps.tile([128, 4], F32, tag="sm", bufs=2)
            nc.tensor.matmul(bc[:], lhsT=hhot, rhs=vals[:], start=True, stop=True)
            bcs = smp.tile([128, 4], F32, tag="bcs", bufs=2)
            nc.vector.tensor_copy(out=bcs[:], in_=bc[:])
            for tt in range(2):
                t = 2 * h0 + tt
                if tt == 0:
                    nc.scalar.activation(
                        out=xn_g[t // 2][:, t % 2, :],
                        in_=x_sb[:, t, :],
                        func=AF.Identity,
                        scale=bcs[:, 2 * tt : 2 * tt + 1],
                        bias=bcs[:, 2 * tt + 1 : 2 * tt + 2],
                    )
                else:
                    nc.vector.tensor_scalar(
                        out=xn_g[t // 2][:, t % 2, :],
                        in0=x_sb[:, t, :],
                        scalar1=bcs[:, 2 * tt : 2 * tt + 1],
                        scalar2=bcs[:, 2 * tt + 1 : 2 * tt + 2],
                        op0=OP.mult,
                        op1=OP.add,
                    )

        qs = [None] * S

        def emit_qm(s):
            """Qm = QS*(M xn) per output tile; evac on ACT adds QS*bb."""
            xn_g = xns[s]
            q_sb = qp.tile([128, 2, 2, N], FP8, tag="q")
            qs[s] = q_sb
            for ot in range(CT):
                psm = ps.tile([128, N], F32, tag="mm2", bufs=3)
                for g in range(2):
                    for ib in range(IB):
                        nc.tensor.matmul(
                            psm[:, ib * IBS : (ib + 1) * IBS],
                            lhsT=w8[:, 0, g, :, ot * 128 : (ot + 1) * 128],
                            rhs=xn_g[g][:, :, ib * IBS : (ib + 1) * IBS],
                            start=(g == 0),
                            stop=(g == 1),
                            perf_mode=DR,
                        )
                if ot % 2 == 0:
                    nc.scalar.activation(
                        out=q_sb[:, ot // 2, ot % 2, :],
                        in_=psm[:],
                        func=AF.Identity,
                        bias=bb_sb[:, ot : ot + 1],
                        scale=1.0,
                    )
                else:
                    nc.vector.tensor_scalar(
                        out=q_sb[:, ot // 2, ot % 2, :],
                        in0=psm[:],
                        scalar1=bb_sb[:, ot : ot + 1],
                        scalar2=None,
                        op0=OP.add,
                    )

        vs = [None] * S
        ess = [None] * S

        def emit_sv(s):
            """Fused S^T/V^T: per (nt, g) one xn stationary serves both."""
            xn_g = xns[s]
            q_sb = qs[s]
            v_sb = vp.tile([128, NT // 2, 2, C], FP8, tag="v")
            vs[s] = v_sb
            es = [
                ep.tile([128, 2, 2, IBS], FP8, tag="e", name=f"e{s}{jg}")
                for jg in range(NT // 2)
            ]
            ess[s] = es
            psm_v = None
            for nt in range(NT):
                psm_s = ps.tile([128, N], F32, tag="mm2", bufs=3)
                if nt % 2 == 0:
                    psm_v = ps.tile(
                        [128, 2, C], F32, tag="mm2", bufs=3, name=f"psmv{s}{nt}"
                    )
                for g in range(2):
                    lw = xn_g[g][:, :, nt * 128 : (nt + 1) * 128]
                    s_mm = None
                    for ib in range(IB):
                        s_mm = nc.tensor.matmul(
                            psm_s[:, ib * IBS : (ib + 1) * IBS],
                            lhsT=lw,
                            rhs=q_sb[:, g, :, ib * IBS : (ib + 1) * IBS],
                            start=(g == 0),
                            stop=(g == 1),
                            perf_mode=DR,
                        )
                    nc.tensor.matmul(
                        psm_v[:, nt % 2, :],
                        lhsT=lw,
                        rhs=w8[:, 1, g, :, :],
                        start=(g == 0),
                        stop=(g == 1),
                        perf_mode=DR,
                    )
                nc.scalar.activation(
                    out=es[nt // 2][:, nt % 2, :, :],
                    in_=psm_s[:],
                    func=AF.Exp,
                    scale=SEXP,
                )
                # V evacuation on DVE, one [128,1024] op per nt pair
                if nt % 2 == 1:
                    nc.vector.tensor_copy(
                        out=v_sb[:, nt // 2, :, :], in_=psm_v[:]
                    )

        def emit_zop(s):
            """Z row-sums, 1/Z, broadcast, O = V@E^T, proj + residual."""
            x_sb = xs[s]
            v_sb = vs[s]
            es = ess[s]
            # Z
            rz = rzp.tile([1, N], BF16, tag="rz")
            for ib in range(IB):
                zps = ps.tile([1, IBS], F32, tag="sm", bufs=2)
                for jg in range(NT // 2):
                    nc.tensor.matmul(
                        zps[:],
                        lhsT=ones8[:, :, 0:1],
                        rhs=es[jg][:, :, ib, :],
                        start=(jg == 0),
                        stop=(jg == NT // 2 - 1),
                        perf_mode=DR,
                    )
                lnz = rzp.tile([1, IBS], F32, tag="lnz", bufs=2)
                nc.scalar.activation(out=lnz[:], in_=zps[:], func=AF.Ln)
                nc.scalar.activation(
                    out=rz[:, ib * IBS : (ib + 1) * IBS], in_=lnz[:],
                    func=AF.Exp, scale=-1.0,
                )
            # broadcast 1/Z across partitions (bf16 K=1 matmul) -> zb
            zb_ps = ps.tile([128, N], F32, tag="mm2", bufs=3)
            for ib in range(IB):
                nc.tensor.matmul(
                    zb_ps[:, ib * IBS : (ib + 1) * IBS],
                    lhsT=ones_row,
                    rhs=rz[:, ib * IBS : (ib + 1) * IBS],
                    start=True,
                    stop=True,
                )
            zb = zbp.tile([128, N], BF16, tag="zb")
            nc.scalar.activation(out=zb[:], in_=zb_ps[:], func=AF.Identity)
            # O = V @ E^T, normalized during evacuation
            o_sb = op_.tile([128, 2, 2, N], FP8, tag="o")
            for ct in range(CT):
                psm_o = ps.tile([128, N], F32, tag="mm2", bufs=3)
                for jg in range(NT // 2):
                    for ib in range(IB):
                        nc.tensor.matmul(
                            psm_o[:, ib * IBS : (ib + 1) * IBS],
                            lhsT=v_sb[:, jg, :, ct * 128 : (ct + 1) * 128],
                            rhs=es[jg][:, :, ib, :],
                            start=(jg == 0),
                            stop=(jg == NT // 2 - 1),
                            perf_mode=DR,
                        )
                nc.vector.tensor_tensor(
                    out=o_sb[:, ct // 2, ct % 2, :], in0=psm_o[:], in1=zb[:],
                    op=OP.mult,
                )
            # proj + pb + residual, one op per output tile, then DMA out
            for ot in range(CT):
                psm_p = ps.tile([128, N], F32, tag="mm2", bufs=3)
                for g in range(2):
                    for ib in range(IB):
                        nc.tensor.matmul(
                            psm_p[:, ib * IBS : (ib + 1) * IBS],
                            lhsT=w8[:, 2, g, :, ot * 128 : (ot + 1) * 128],
                            rhs=o_sb[:, g, :, ib * IBS : (ib + 1) * IBS],
                            start=(g == 0),
                            stop=(g == 1),
                            perf_mode=DR,
                        )
                ob = obp.tile([128, N], BF16, tag="ob")
                nc.vector.scalar_tensor_tensor(
                    out=ob[:],
                    in0=psm_p[:],
                    scalar=pb[:, ot : ot + 1],
                    in1=x_sb[:, ot, :],
                    op0=OP.add,
                    op1=OP.add,
                )
                for hh in range(2):
                    eng = nc.sync if (2 * ot + hh) % 2 == 0 else nc.scalar
                    eng.dma_start(
                        out=out_ap[s][:, ot, hh * IBS : (hh + 1) * IBS],
                        in_=ob[:, hh * IBS : (hh + 1) * IBS],
                    )

        emit_gn_stats(0, 0)
        emit_gn_chain(0, 0)
        emit_gn_stats(0, 1)
        emit_gn_chain(0, 1)
        emit_qm(0)
        emit_gn_stats(1, 0)
        emit_gn_stats(1, 1)
        emit_sv(0)
        emit_gn_chain(1, 0)
        emit_gn_chain(1, 1)
        emit_qm(1)
        emit_sv(1)
        emit_zop(0)
        emit_zop(1)

    _dedup_ldweights(nc)
    _split_excess_waits(nc)
    return nc


_NC = None


def kernel(x, norm_w, norm_b, qkv_w, qkv_b, proj_w, proj_b):
    global _NC, LAST_RESULT
    x = np.ascontiguousarray(np.asarray(x, dtype=np.float32))
    norm_w = np.asarray(norm_w, dtype=np.float32)
    norm_b = np.asarray(norm_b, dtype=np.float32)
    qkv_w = np.asarray(qkv_w, dtype=np.float32)
    qkv_b = np.asarray(qkv_b, dtype=np.float32)
    proj_w = np.asarray(proj_w, dtype=np.float32)
    proj_b = np.asarray(proj_b, dtype=np.float32)

    # fold GroupNorm affine into qkv
    wq_full = qkv_w * norm_w[None, :]
    bq_full = qkv_b + qkv_w @ norm_b
    wq_, wk_, wv_ = wq_full[0:C], wq_full[C : 2 * C], wq_full[2 * C : 3 * C]
    bq_, bk_, bv_ = bq_full[0:C], bq_full[C : 2 * C], bq_full[2 * C : 3 * C]
    del bk_  # cancels in softmax (per-i and constant terms)
    pb_ = proj_w @ bv_ + proj_b
    m_ = wk_.T @ wq_  # S^T = xn^T (M xn + bb); QS scales into fp8 range
    bb_ = wk_.T @ bq_

    def wtile(w):  # [o, c] -> DoubleRow lhsT planes [128, 2(g), 2(q), o]
        return w.T.reshape(2, 2, 128, C).transpose(2, 0, 1, 3)

    def btile(b):  # [C] -> [128, ct]
        return b.reshape(CT, 128).T

    wall = np.ascontiguousarray(
        np.stack(
            [wtile(m_ * QS), wtile(wv_), wtile(proj_w)], axis=1
        ).reshape(128, 12, C).astype(ml_dtypes.float8_e4m3)
    )
    ball = np.ascontiguousarray(
        np.concatenate([btile(bb_ * QS), btile(pb_)], axis=1).astype(np.float32)
    )
    cl = np.arange(128)
    ghot = np.zeros((128, 2), np.float32)
    ghot[cl, cl // 64] = 1.0 / 64.0
    hhot = np.zeros((2, 128), np.float32)
    hhot[cl // 64, cl] = 1.0

    common = {
        "wall": wall,
        "ball": ball,
        "ca": ghot.astype(ml_dtypes.bfloat16),
        "cb": hhot.astype(ml_dtypes.bfloat16),
    }
    xr = x.reshape(NCORES, S, C, N)
    in_maps = [dict(common, x=np.ascontiguousarray(xr[i])) for i in range(NCORES)]

    if _NC is None:
        _NC = _build()
    res = run_bass_kernel_spmd(
        _NC, in_maps, core_ids=list(range(NCORES)), trace=TRACE
    )
    LAST_RESULT = res
    out = np.stack([res.results[i]["out"] for i in range(NCORES)])
    return np.ascontiguousarray(out.reshape(B, C, 32, 32).astype(np.float32))
